# revision 14
# baseline (speedup 1.0000x reference)
"""Trainium2 Bass kernel for nn_MCLSTM_65386582114595.

MC-LSTM-style recurrence: B=1024 batch, T=366 days, 365 sequential steps.
Per step: fused matmul [C_t | aux_t] @ W_cat (576 redis + 24 part + 24 cons
+ 1 assim outputs), softmax gates, and per-sample mass redistribution
C2[b,j] = sum_i (C'[b,i]/S[b,i]) * exp(Z[b,i,j]).

Sharding: pure data parallel, batch 1024 -> 8 cores x 128 (= partition dim).

Device layout (per core):
  - stage_C SBUF tile [128, 366*32]: slot t = [C_t(24) | pad | aux8_t(8)]
    aux8 = [dvs, rad_h, rad_l, tmax, tmin, N_cum, ones, cpot] (host-prepped;
    ones-row folds biases into the matmul, cpot read as a per-partition
    scalar, rad split hi/lo so a bf16 matmul path stays accurate).
  - per step: PE transpose of the slot -> xT [32,128]; fp32 matmuls
    (512-col redis chunk + 113-col chunk w/ redis-tail+gates); ACT exp;
    DVE group reductions via 3D access patterns; the redis softmax skips
    max-subtraction (|z_redis| <= ~5 by construction: C tiny, rad in [0,1));
    the part softmax is stabilized with a per-row max via ACT bias.
  - gate2 (output-only convergence pass) identical structure, off the
    recurrence critical path.
"""
import sys

sys.path.insert(0, "/opt/trn_rl_repo")

import numpy as np

B, T = 1024, 366
NCORES = 8
PB = B // NCORES            # 128 batch rows per core
C_DIM, AUX_DIM, IN_DIM = 24, 5, 29
NSTEP = T - 1               # 365
SLOT = 40                   # stage slot width (24 C + 8 pad + 8 aux)
NZA = 512                   # redis chunk A columns
NZB = 113                   # 64 redis tail + 24 part + 24 cons + 1 assim
LEA = STE = GRA = 8

# aux8 column indices (within slot cols 32..39)
A_DVS, A_RADH, A_RADL, A_TMAX, A_TMIN, A_NCUM, A_ONE, A_CPOT = range(8)


def _build_bass(nstep=NSTEP, mm_mode="fp32", run_steps=None, repeat=1):
    import os as _os
    G2_PRIO = int(_os.environ.get("G2_PRIO", "25"))
    import concourse.bass as bass
    import concourse.tile as tile
    from concourse import bacc, mybir

    f32 = mybir.dt.float32
    bf16 = mybir.dt.bfloat16
    AF = mybir.ActivationFunctionType
    OP = mybir.AluOpType
    AX = mybir.AxisListType

    nc = bacc.Bacc("TRN2", target_bir_lowering=False, debug=False, num_devices=1)

    aux_d = nc.dram_tensor("aux8", [PB, T, 8], f32, kind="ExternalInput").ap()
    zgaux_d = nc.dram_tensor("zgaux", [PB, nstep, 49], f32, kind="ExternalInput").ap()
    wah_d = nc.dram_tensor("wah", [SLOT, NZA], f32, kind="ExternalInput").ap()
    wal_d = nc.dram_tensor("wal", [SLOT, NZA], f32, kind="ExternalInput").ap()
    wbh_d = nc.dram_tensor("wbh", [SLOT, NZB], f32, kind="ExternalInput").ap()
    wbl_d = nc.dram_tensor("wbl", [SLOT, NZB], f32, kind="ExternalInput").ap()
    id_d = nc.dram_tensor("ident", [PB, PB], f32, kind="ExternalInput").ap()
    ccell_d = nc.dram_tensor("ccell", [PB, nstep, C_DIM], f32, kind="ExternalOutput").ap()
    cconv_d = nc.dram_tensor("cconv", [PB, nstep, C_DIM], f32, kind="ExternalOutput").ap()

    hilo = mm_mode == "hilo"

    with tile.TileContext(nc) as tc:
        with (
            tc.tile_pool(name="big", bufs=1) as big,
            tc.tile_pool(name="consts", bufs=1) as consts,
            tc.tile_pool(name="xt", bufs=3) as sbx,
            tc.tile_pool(name="xbh", bufs=3) as sbxh,
            tc.tile_pool(name="e1", bufs=2) as sbe1,
            tc.tile_pool(name="e2", bufs=2) as sbe2,
            tc.tile_pool(name="p1", bufs=2) as sbp1,
            tc.tile_pool(name="p2", bufs=2) as sbp2,
            tc.tile_pool(name="small", bufs=4) as small,
            tc.tile_pool(name="ps_xt", bufs=3, space="PSUM") as ps_xt,
            tc.tile_pool(name="ps_za", bufs=2, space="PSUM") as ps_za,
            tc.tile_pool(name="ps_zb", bufs=1, space="PSUM") as ps_zb,
            tc.tile_pool(name="ps_z2a", bufs=1, space="PSUM") as ps_z2a,
            tc.tile_pool(name="ps_z2b", bufs=1, space="PSUM") as ps_z2b,
        ):
            # ---- constants / preload ----
            ident = consts.tile([PB, PB], f32)
            nc.sync.dma_start(ident[:], id_d[:])
            wah = consts.tile([SLOT, NZA], bf16)
            wal = consts.tile([SLOT, NZA], bf16)
            wbh = consts.tile([SLOT, NZB], bf16)
            wbl = consts.tile([SLOT, NZB], bf16)
            wah32 = consts.tile([SLOT, NZA], f32)
            wal32 = consts.tile([SLOT, NZA], f32)
            wbh32 = consts.tile([SLOT, NZB], f32)
            wbl32 = consts.tile([SLOT, NZB], f32)
            nc.sync.dma_start(wah32[:], wah_d[:])
            nc.sync.dma_start(wal32[:], wal_d[:])
            nc.sync.dma_start(wbh32[:], wbh_d[:])
            nc.sync.dma_start(wbl32[:], wbl_d[:])
            nc.scalar.copy(wah[:], wah32[:])
            nc.scalar.copy(wal[:], wal32[:])
            nc.scalar.copy(wbh[:], wbh32[:])
            nc.scalar.copy(wbl[:], wbl32[:])

            stage = big.tile([PB, (nstep + 1) * SLOT], f32)
            sv3 = stage[:].rearrange("p (t s) -> p t s", s=SLOT)
            stagev = big.tile([PB, nstep * C_DIM], f32)

            def sv(t, a, b):
                return stage[:, t * SLOT + a:t * SLOT + b]

            def vvs(t):
                return stagev[:, t * C_DIM:(t + 1) * C_DIM]

            # aux preload into slot cols 32..40; zero C_0 and pad cols
            nc.sync.dma_start(sv3[:, :, 32:SLOT], aux_d[:, 0:nstep + 1, :])
            nc.vector.memset(stage[:, 0:C_DIM], 0.0)
            nc.vector.memset(sv3[:, :, 24:32], 0.0)

            nrun = nstep if run_steps is None else run_steps

            def make_xt(t):
                xt_ps = ps_xt.tile([SLOT, PB], f32, tag="xtps")
                nc.tensor.transpose(xt_ps[:], sv(t, 0, SLOT), ident[:])
                return xt_ps

            def gate2(t, xt_prev, xt_cur):
                """C_conv for step t: x2 = [C_{t+1} (rows 0..23 of xt_cur),
                aux_t (rows 32..39 of xt_prev)]."""
                x2bh = sbxh.tile([SLOT, PB], bf16, tag="x2bh")
                nc.scalar.copy(x2bh[0:32, :], xt_cur[0:32, :])
                nc.scalar.copy(x2bh[32:SLOT, :], xt_prev[32:SLOT, :])
                z2A = ps_z2a.tile([PB, NZA], f32, tag="z2A")
                nc.tensor.matmul(z2A[:], x2bh[:], wah[:], start=True, stop=False)
                nc.tensor.matmul(z2A[:], x2bh[:], wal[:], start=False, stop=True)
                z2B = ps_z2b.tile([PB, 64], f32, tag="z2B")
                nc.tensor.matmul(z2B[:], x2bh[:], wbh[:, 0:64], start=True, stop=False)
                nc.tensor.matmul(z2B[:], x2bh[:], wbl[:, 0:64], start=False, stop=True)

                E2 = sbe2.tile([PB, 576], f32, tag="E2")
                nc.scalar.activation(E2[:, 0:256], z2A[:, 0:256], AF.Exp)
                nc.scalar.activation(E2[:, 256:NZA], z2A[:, 256:NZA], AF.Exp)
                nc.scalar.activation(E2[:, NZA:576], z2B[:], AF.Exp)

                S2 = small.tile([PB, C_DIM], f32, tag="S2")
                nc.vector.tensor_reduce(S2[:], E2[:].rearrange("p (i j) -> p i j", i=C_DIM), axis=AX.X, op=OP.add)
                rS2 = small.tile([PB, C_DIM], f32, tag="rS2")
                nc.vector.reciprocal_approx_fast(rS2[:], S2[:])
                w2 = small.tile([PB, C_DIM], f32, tag="w2")
                nc.gpsimd.tensor_tensor(w2[:], sv(t + 1, 0, C_DIM), rS2[:], op=OP.mult)
                P2 = sbp2.tile([PB, 576], f32, tag="P2")
                nc.gpsimd.tensor_tensor(
                    P2[:].rearrange("p (i j) -> p i j", i=C_DIM),
                    E2[:].rearrange("p (i j) -> p i j", i=C_DIM),
                    w2[:].unsqueeze(2).broadcast_to([PB, C_DIM, C_DIM]),
                    op=OP.mult,
                )
                # sum over i via gpsimd pairwise tree (keeps DVE free)
                pv = P2[:].rearrange("p (i j) -> p i j", i=C_DIM)
                r1 = sbp2.tile([PB, 12 * C_DIM], f32, tag="r1")
                r1v = r1[:].rearrange("p (i j) -> p i j", i=12)
                nc.gpsimd.tensor_tensor(r1v, pv[:, 0:12, :], pv[:, 12:24, :], op=OP.add)
                r2 = small.tile([PB, 6 * C_DIM], f32, tag="r2")
                r2v = r2[:].rearrange("p (i j) -> p i j", i=6)
                nc.gpsimd.tensor_tensor(r2v, r1v[:, 0:6, :], r1v[:, 6:12, :], op=OP.add)
                r3 = small.tile([PB, 3 * C_DIM], f32, tag="r3")
                r3v = r3[:].rearrange("p (i j) -> p i j", i=3)
                nc.gpsimd.tensor_tensor(r3v, r2v[:, 0:3, :], r2v[:, 3:6, :], op=OP.add)
                r4 = small.tile([PB, C_DIM], f32, tag="r4")
                nc.gpsimd.tensor_tensor(r4[:], r3[:, 0:C_DIM], r3[:, C_DIM:2 * C_DIM], op=OP.add)
                nc.gpsimd.tensor_tensor(vvs(t), r4[:], r3[:, 2 * C_DIM:3 * C_DIM], op=OP.add)

            xt_prev = None
            for t in [tt for _ in range(repeat) for tt in range(nrun)]:
                # ======== gate 1 (recurrence) ========
                xt_ps = make_xt(t)
                # prefetch the host-precomputed gates-aux logits
                zga = small.tile([PB, 49], f32, tag="zga")
                nc.sync.dma_start(zga[:], zgaux_d[:, t, :])

                zA = ps_za.tile([PB, NZA], f32, tag="zA")
                xbh = sbxh.tile([SLOT, PB], bf16, tag="xbh")
                nc.scalar.copy(xbh[:], xt_ps[:])
                nc.tensor.matmul(zA[:], xbh[:], wah[:], start=True, stop=False)
                nc.tensor.matmul(zA[:], xbh[:], wal[:], start=False, stop=True)
                zB = ps_zb.tile([PB, NZB], f32, tag="zB")
                nc.tensor.matmul(zB[:], xbh[:], wbh[:], start=True, stop=False)
                nc.tensor.matmul(zB[:], xbh[:], wbl[:], start=False, stop=True)

                E1 = sbe1.tile([PB, 576], f32, tag="E1")
                nc.scalar.activation(E1[:, 0:NZA], zA[:], AF.Exp)
                nc.scalar.activation(E1[:, NZA:576], zB[:, 0:64], AF.Exp)

                # gates: z = C-part (psum) + host aux logits
                zg = small.tile([PB, 49], f32, tag="zg")
                nc.vector.tensor_tensor(zg[:], zB[:, 64:NZB], zga[:], op=OP.add)
                nm = small.tile([PB, 1], f32, tag="nm")
                nc.vector.tensor_reduce(nm[:], zg[:, 0:C_DIM], axis=AX.X, op=OP.max, negate=True)
                Ep = small.tile([PB, C_DIM], f32, tag="Ep")
                Sp = small.tile([PB, 1], f32, tag="Sp")
                nc.scalar.activation(Ep[:], zg[:, 0:C_DIM], AF.Exp, bias=nm[:], accum_out=Sp[:])
                # cons cols pre-negated on host: one tanh covers
                # th = tanh(-zc/2) (cols 24..48) and tha = tanh(za/2) (col 48)
                thall = small.tile([PB, 25], f32, tag="thall")
                nc.scalar.activation(thall[:], zg[:, C_DIM:49], AF.Tanh, scale=0.5)
                th = thall[:, 0:C_DIM]
                tha = thall[:, C_DIM:25]

                # small chain kept on DVE, emitted before S1 so the gates
                # branch completes while the big reduce runs
                rSp = small.tile([PB, 1], f32, tag="rSp")
                nc.vector.reciprocal(rSp[:], Sp[:])
                acp = small.tile([PB, 1], f32, tag="acp")
                nc.vector.tensor_scalar(acp[:], tha, 1.0, sv(t, 32 + A_CPOT, 32 + A_CPOT + 1), op0=OP.add, op1=OP.mult)
                acprS = small.tile([PB, 1], f32, tag="acprS")
                nc.vector.tensor_scalar(acprS[:], acp[:], rSp[:], None, op0=OP.mult)
                Cpre = small.tile([PB, C_DIM], f32, tag="Cpre")
                nc.vector.scalar_tensor_tensor(Cpre[:], Ep[:], acprS[:], sv(t, 0, C_DIM), op0=OP.mult, op1=OP.add)
                Cp2 = small.tile([PB, C_DIM], f32, tag="Cp2")
                nc.vector.scalar_tensor_tensor(Cp2[:], th, 1.0, Cpre[:], op0=OP.add, op1=OP.mult)

                S1 = small.tile([PB, C_DIM], f32, tag="S1")
                nc.vector.tensor_reduce(S1[:], E1[:].rearrange("p (i j) -> p i j", i=C_DIM), axis=AX.X, op=OP.add)
                rS1 = small.tile([PB, C_DIM], f32, tag="rS1")
                nc.vector.reciprocal_approx_fast(rS1[:], S1[:])
                w1 = small.tile([PB, C_DIM], f32, tag="w1")
                nc.vector.scalar_tensor_tensor(w1[:], Cp2[:], 0.5, rS1[:], op0=OP.mult, op1=OP.mult)
                P1 = sbp1.tile([PB, 576], f32, tag="P1")
                nc.vector.tensor_tensor(
                    P1[:].rearrange("p (i j) -> p i j", i=C_DIM),
                    E1[:].rearrange("p (i j) -> p i j", i=C_DIM),
                    w1[:].unsqueeze(2).broadcast_to([PB, C_DIM, C_DIM]),
                    op=OP.mult,
                )
                nc.vector.tensor_reduce(
                    sv(t + 1, 0, C_DIM),
                    P1[:].rearrange("p (i j) -> p j i", i=C_DIM),
                    axis=AX.X, op=OP.add,
                )

                if xt_prev is not None:
                    with tc.high_priority(offset=-G2_PRIO):
                        gate2(max(t - 1, 0), xt_prev, xt_ps)
                xt_prev = xt_ps

            # last gate2 needs the transpose of the final slot
            xt_last = make_xt(nrun)
            with tc.high_priority(offset=-G2_PRIO):
                gate2(nrun - 1, xt_prev, xt_last)

            # outputs
            nc.sync.dma_start(ccell_d[:], sv3[:, 1:nstep + 1, 0:C_DIM])
            nc.sync.dma_start(cconv_d[:], stagev[:])

    nc.compile()
    return nc


# ---------------- host side ----------------

def _round_bf16(x):
    import ml_dtypes
    return x.astype(ml_dtypes.bfloat16).astype(np.float32)


def _host_prep(inputs, mm_mode):
    X = np.asarray(inputs["X"], np.float32)
    ORY = np.asarray(inputs["ORY"], np.float32)
    W_redis = np.asarray(inputs["W_redis"], np.float32)
    b_redis = np.asarray(inputs["b_redis"], np.float32)
    W_part = np.asarray(inputs["W_part"], np.float32)
    b_part = np.asarray(inputs["b_part"], np.float32)
    W_cons = np.asarray(inputs["W_cons"], np.float32)
    b_cons = np.asarray(inputs["b_cons"], np.float32)
    W_assim = np.asarray(inputs["W_assim"], np.float32)
    b_assim = np.asarray(inputs["b_assim"], np.float32)

    # preprocessing (mirrors reference, fp32)
    N_cum = np.cumsum(X[:, :, 3], axis=-1, dtype=np.float32)[..., None]
    t_ave = ((X[:, :, 1:2] + X[:, :, 2:3]) * 0.5).astype(np.float32)
    dvs = ORY[:, :, 0:1]
    rad = X[:, :, 0:1]
    tmax = X[:, :, 1:2]
    tmin = X[:, :, 2:3]
    FRPAR = np.float32(0.5)
    eff = (np.float32(0.54)
           - (np.clip(t_ave * np.float32(50.0), np.float32(10.0), np.float32(40.0))
              - np.float32(10.0)) / np.float32(30.0) * np.float32(0.54 - 0.36))
    R2C = (np.float32(40000.0 / 20000.0) * FRPAR * eff / np.float32(3.6)
           * np.float32(12.0 / 44.0))
    cpot = (rad * R2C).astype(np.float32)

    rad_h = _round_bf16(rad)
    rad_l = (rad - rad_h).astype(np.float32)

    aux8 = np.zeros((B, T, 8), np.float32)
    aux8[:, :, A_DVS] = dvs[:, :, 0]
    aux8[:, :, A_RADH] = rad_h[:, :, 0]
    aux8[:, :, A_RADL] = rad_l[:, :, 0]
    aux8[:, :, A_TMAX] = tmax[:, :, 0]
    aux8[:, :, A_TMIN] = tmin[:, :, 0]
    aux8[:, :, A_NCUM] = N_cum[:, :, 0]
    aux8[:, :, A_ONE] = 1.0
    aux8[:, :, A_CPOT] = cpot[:, :, 0] * 0.5

    # fused weight matrices, row layout = x rows:
    # [C(24), pad(8), dvs, rad_h, rad_l, tmax, tmin, N_cum, ones, cpot/2]
    def rowstack(Wt, bias, masked):
        out = np.zeros((SLOT, Wt.shape[1]), np.float32)
        out[0:C_DIM] = Wt[0:C_DIM]
        if masked:
            out[32 + A_RADH] = Wt[25]
            out[32 + A_RADL] = Wt[25]
        else:
            out[32 + A_DVS] = Wt[24]
            out[32 + A_RADH] = Wt[25]
            out[32 + A_RADL] = Wt[25]
            out[32 + A_TMAX] = Wt[26]
            out[32 + A_TMIN] = Wt[27]
            out[32 + A_NCUM] = Wt[28]
        out[32 + A_ONE] = bias
        return out

    Wr = rowstack(W_redis.T, b_redis, masked=True)          # [40, 576]

    # gates: C-part on device (rows 0..23), aux part precomputed on host.
    # cons block negated so one tanh(0.5*z) op serves cons' and assim.
    Wg = np.concatenate([W_part.T, -W_cons.T, W_assim.T], axis=1)  # [29, 49]
    WgC = np.zeros((SLOT, 49), np.float32)
    WgC[0:C_DIM] = Wg[0:C_DIM]
    bg = np.concatenate([b_part, -b_cons, b_assim]).astype(np.float64)
    # aux logits: [dvs, rad, tmax, tmin, N_cum] @ Wg_aux + bias (float64)
    auxfull = np.stack([dvs[:, :, 0], rad[:, :, 0], tmax[:, :, 0],
                        tmin[:, :, 0], N_cum[:, :, 0]], axis=-1).astype(np.float64)
    Zgaux = (auxfull @ Wg[C_DIM:IN_DIM].astype(np.float64) + bg).astype(np.float32)
    Zgaux = np.ascontiguousarray(Zgaux[:, 0:T - 1, :])      # [B, 365, 49]

    WA = np.ascontiguousarray(Wr[:, 0:NZA])                 # [40, 512]
    WBtail = np.ascontiguousarray(Wr[:, NZA:576])           # [40, 64]
    WB = np.concatenate([WBtail, WgC], axis=1)              # [40, 113]

    WAh = _round_bf16(WA)
    WAl = _round_bf16(WA - WAh)
    WBh = _round_bf16(WB)
    WBl = _round_bf16(WB - WBh)
    ident = np.eye(PB, dtype=np.float32)
    return dict(aux8=aux8, zgaux=Zgaux, wah=WAh, wal=WAl, wbh=WBh, wbl=WBl,
                ident=ident)


_CACHE = {}


def _get_bass(mm_mode):
    key = (NSTEP, mm_mode)
    if key not in _CACHE:
        _CACHE[key] = _build_bass(NSTEP, mm_mode)
    return _CACHE[key]


def kernel(**inputs):
    from concourse.bass_utils import run_bass_kernel_spmd

    mm_mode = inputs.pop("_mm_mode", "fp32")
    trace = inputs.pop("_trace", False)

    hp = _host_prep(inputs, mm_mode)
    nc = _get_bass(mm_mode)

    in_maps = []
    for c in range(NCORES):
        sl = slice(c * PB, (c + 1) * PB)
        m = dict(hp)
        m["aux8"] = np.ascontiguousarray(hp["aux8"][sl])
        m["zgaux"] = np.ascontiguousarray(hp["zgaux"][sl])
        in_maps.append(m)

    import concourse.mybir as _mybir
    expected = set()
    for alloc in nc.m.functions[0].allocations:
        if isinstance(alloc, _mybir.MemoryLocationSet) and alloc.kind == "ExternalInput":
            expected.add(alloc.memorylocations[0].name)
    in_maps = [{k: v for k, v in m.items() if k in expected} for m in in_maps]
    res = run_bass_kernel_spmd(nc, in_maps, core_ids=list(range(NCORES)),
                               trace=trace)
    ccell = np.concatenate([r["ccell"] for r in res.results], axis=0)  # [B, 365, 24]
    cconv = np.concatenate([r["cconv"] for r in res.results], axis=0)

    # host postprocessing: summary channels + all_day assembly
    ORY = np.asarray(inputs["ORY"], np.float32)
    c2a = np.asarray(inputs["c2a_par"], np.float32)
    g2y = np.asarray(inputs["g2y_par"], np.float32)
    pai = np.abs(ccell * c2a).sum(2, keepdims=True)
    lea = ccell[:, :, 0:LEA].sum(2, keepdims=True) / np.float32(0.419)
    ste = ccell[:, :, LEA:LEA + STE].sum(2, keepdims=True) / np.float32(0.431)
    gra = ccell[:, :, LEA + STE:C_DIM].sum(2, keepdims=True) / np.float32(0.487)
    agb = lea + ste + gra
    yie = np.abs(ccell[:, :, LEA + STE:C_DIM] * g2y).sum(2, keepdims=True) / np.float32(0.487)
    dvs = ORY[:, :, 0:1]
    all_day = np.concatenate([dvs[:, 1:], pai, lea, ste, gra, agb, yie], axis=2)
    all_day = np.concatenate([ORY[:, 0:1, :], all_day], axis=1)
    if "_results_holder" in inputs:
        inputs["_results_holder"].append(res)
    return all_day.astype(np.float32), ccell.astype(np.float32), cconv.astype(np.float32)


# revision 15
# speedup vs baseline: 1.3216x; 1.3216x over previous
"""Trainium2 Bass kernel for nn_MCLSTM_65386582114595.

MC-LSTM-style recurrence: B=1024 batch, T=366 days, 365 sequential steps.
Per step: fused matmul [C_t | aux_t] @ W_cat (576 redis + 24 part + 24 cons
+ 1 assim outputs), softmax gates, and per-sample mass redistribution
C2[b,j] = sum_i (C'[b,i]/S[b,i]) * exp(Z[b,i,j]).

Sharding: pure data parallel, batch 1024 -> 8 cores x 128 (= partition dim).

Device layout (per core):
  - stage_C SBUF tile [128, 366*32]: slot t = [C_t(24) | pad | aux8_t(8)]
    aux8 = [dvs, rad_h, rad_l, tmax, tmin, N_cum, ones, cpot] (host-prepped;
    ones-row folds biases into the matmul, cpot read as a per-partition
    scalar, rad split hi/lo so a bf16 matmul path stays accurate).
  - per step: PE transpose of the slot -> xT [32,128]; fp32 matmuls
    (512-col redis chunk + 113-col chunk w/ redis-tail+gates); ACT exp;
    DVE group reductions via 3D access patterns; the redis softmax skips
    max-subtraction (|z_redis| <= ~5 by construction: C tiny, rad in [0,1));
    the part softmax is stabilized with a per-row max via ACT bias.
  - gate2 (output-only convergence pass) identical structure, off the
    recurrence critical path.
"""
import sys

sys.path.insert(0, "/opt/trn_rl_repo")

import numpy as np

B, T = 1024, 366
NCORES = 8
PB = B // NCORES            # 128 batch rows per core
C_DIM, AUX_DIM, IN_DIM = 24, 5, 29
NSTEP = T - 1               # 365
SLOT = 40                   # stage slot width (24 C + 8 pad + 8 aux)
NZA = 512                   # redis chunk A columns
NZB = 113                   # 64 redis tail + 24 part + 24 cons + 1 assim
LEA = STE = GRA = 8

# aux8 column indices (within slot cols 32..39)
A_DVS, A_RADH, A_RADL, A_TMAX, A_TMIN, A_NCUM, A_ONE, A_CPOT = range(8)


def _build_bass(nstep=NSTEP, mm_mode="hilo", run_steps=None, repeat=1):
    import os as _os
    G2_PRIO = int(_os.environ.get("G2_PRIO", "25"))
    import concourse.bass as bass
    import concourse.tile as tile
    from concourse import bacc, mybir

    f32 = mybir.dt.float32
    bf16 = mybir.dt.bfloat16
    AF = mybir.ActivationFunctionType
    OP = mybir.AluOpType
    AX = mybir.AxisListType

    nc = bacc.Bacc("TRN2", target_bir_lowering=False, debug=False, num_devices=1)

    aux_d = nc.dram_tensor("aux8", [PB, T, 8], f32, kind="ExternalInput").ap()
    zgaux_d = nc.dram_tensor("zgaux", [PB, nstep, 49], f32, kind="ExternalInput").ap()
    wah_d = nc.dram_tensor("wah", [SLOT, NZA], f32, kind="ExternalInput").ap()
    wal_d = nc.dram_tensor("wal", [SLOT, NZA], f32, kind="ExternalInput").ap()
    wbh_d = nc.dram_tensor("wbh", [SLOT, NZB], f32, kind="ExternalInput").ap()
    wbl_d = nc.dram_tensor("wbl", [SLOT, NZB], f32, kind="ExternalInput").ap()
    id_d = nc.dram_tensor("ident", [PB, PB], f32, kind="ExternalInput").ap()
    ccell_d = nc.dram_tensor("ccell", [PB, nstep, C_DIM], f32, kind="ExternalOutput").ap()
    cconv_d = nc.dram_tensor("cconv", [PB, nstep, C_DIM], f32, kind="ExternalOutput").ap()

    hilo = mm_mode == "hilo"

    with tile.TileContext(nc) as tc:
        with (
            tc.tile_pool(name="big", bufs=1) as big,
            tc.tile_pool(name="consts", bufs=1) as consts,
            tc.tile_pool(name="xt", bufs=3) as sbx,
            tc.tile_pool(name="xbh", bufs=3) as sbxh,
            tc.tile_pool(name="e1", bufs=2) as sbe1,
            tc.tile_pool(name="e2", bufs=2) as sbe2,
            tc.tile_pool(name="p1", bufs=2) as sbp1,
            tc.tile_pool(name="p2", bufs=2) as sbp2,
            tc.tile_pool(name="small", bufs=4) as small,
            tc.tile_pool(name="ps_xt", bufs=3, space="PSUM") as ps_xt,
            tc.tile_pool(name="ps_za", bufs=2, space="PSUM") as ps_za,
            tc.tile_pool(name="ps_zb", bufs=1, space="PSUM") as ps_zb,
            tc.tile_pool(name="ps_z2a", bufs=1, space="PSUM") as ps_z2a,
            tc.tile_pool(name="ps_z2b", bufs=1, space="PSUM") as ps_z2b,
        ):
            # ---- constants / preload ----
            ident = consts.tile([PB, PB], f32)
            nc.sync.dma_start(ident[:], id_d[:])
            wah = consts.tile([SLOT, NZA], bf16)
            wal = consts.tile([SLOT, NZA], bf16)
            wbh = consts.tile([SLOT, NZB], bf16)
            wbl = consts.tile([SLOT, NZB], bf16)
            wah32 = consts.tile([SLOT, NZA], f32)
            wal32 = consts.tile([SLOT, NZA], f32)
            wbh32 = consts.tile([SLOT, NZB], f32)
            wbl32 = consts.tile([SLOT, NZB], f32)
            nc.sync.dma_start(wah32[:], wah_d[:])
            nc.sync.dma_start(wal32[:], wal_d[:])
            nc.sync.dma_start(wbh32[:], wbh_d[:])
            nc.sync.dma_start(wbl32[:], wbl_d[:])
            nc.scalar.copy(wah[:], wah32[:])
            nc.scalar.copy(wal[:], wal32[:])
            nc.scalar.copy(wbh[:], wbh32[:])
            nc.scalar.copy(wbl[:], wbl32[:])

            stage = big.tile([PB, (nstep + 1) * SLOT], f32)
            sv3 = stage[:].rearrange("p (t s) -> p t s", s=SLOT)
            stagev = big.tile([PB, nstep * C_DIM], f32)

            def sv(t, a, b):
                return stage[:, t * SLOT + a:t * SLOT + b]

            def vvs(t):
                return stagev[:, t * C_DIM:(t + 1) * C_DIM]

            # aux preload into slot cols 32..40; zero C_0 and pad cols
            nc.sync.dma_start(sv3[:, :, 32:SLOT], aux_d[:, 0:nstep + 1, :])
            nc.vector.memset(stage[:, 0:C_DIM], 0.0)
            nc.vector.memset(sv3[:, :, 24:32], 0.0)

            nrun = nstep if run_steps is None else run_steps

            def make_xt(t):
                xt_ps = ps_xt.tile([SLOT, PB], f32, tag="xtps")
                nc.tensor.transpose(xt_ps[:], sv(t, 0, SLOT), ident[:])
                return xt_ps

            def gate2(t, xt_prev, xt_cur):
                """C_conv for step t: x2 = [C_{t+1} (rows 0..23 of xt_cur),
                aux_t (rows 32..39 of xt_prev)]."""
                x2bh = sbxh.tile([SLOT, PB], bf16, tag="x2bh")
                nc.scalar.copy(x2bh[0:32, :], xt_cur[0:32, :])
                nc.scalar.copy(x2bh[32:SLOT, :], xt_prev[32:SLOT, :])
                z2A = ps_z2a.tile([PB, NZA], f32, tag="z2A")
                nc.tensor.matmul(z2A[:], x2bh[:], wah[:], start=True, stop=False)
                nc.tensor.matmul(z2A[:], x2bh[:], wal[:], start=False, stop=True)
                z2B = ps_z2b.tile([PB, 64], f32, tag="z2B")
                nc.tensor.matmul(z2B[:], x2bh[:], wbh[:, 0:64], start=True, stop=False)
                nc.tensor.matmul(z2B[:], x2bh[:], wbl[:, 0:64], start=False, stop=True)

                E2 = sbe2.tile([PB, 576], f32, tag="E2")
                nc.scalar.activation(E2[:, 0:256], z2A[:, 0:256], AF.Exp)
                nc.scalar.activation(E2[:, 256:NZA], z2A[:, 256:NZA], AF.Exp)
                nc.scalar.activation(E2[:, NZA:576], z2B[:], AF.Exp)

                S2 = small.tile([PB, C_DIM], f32, tag="S2")
                nc.vector.tensor_reduce(S2[:], E2[:].rearrange("p (i j) -> p i j", i=C_DIM), axis=AX.X, op=OP.add)
                rS2 = small.tile([PB, C_DIM], f32, tag="rS2")
                nc.vector.reciprocal_approx_fast(rS2[:], S2[:])
                w2 = small.tile([PB, C_DIM], f32, tag="w2")
                nc.gpsimd.tensor_tensor(w2[:], sv(t + 1, 0, C_DIM), rS2[:], op=OP.mult)
                P2 = sbp2.tile([PB, 576], f32, tag="P2")
                nc.gpsimd.tensor_tensor(
                    P2[:].rearrange("p (i j) -> p i j", i=C_DIM),
                    E2[:].rearrange("p (i j) -> p i j", i=C_DIM),
                    w2[:].unsqueeze(2).broadcast_to([PB, C_DIM, C_DIM]),
                    op=OP.mult,
                )
                # sum over i via gpsimd pairwise tree (keeps DVE free)
                pv = P2[:].rearrange("p (i j) -> p i j", i=C_DIM)
                r1 = sbp2.tile([PB, 12 * C_DIM], f32, tag="r1")
                r1v = r1[:].rearrange("p (i j) -> p i j", i=12)
                nc.gpsimd.tensor_tensor(r1v, pv[:, 0:12, :], pv[:, 12:24, :], op=OP.add)
                r2 = small.tile([PB, 6 * C_DIM], f32, tag="r2")
                r2v = r2[:].rearrange("p (i j) -> p i j", i=6)
                nc.gpsimd.tensor_tensor(r2v, r1v[:, 0:6, :], r1v[:, 6:12, :], op=OP.add)
                r3 = small.tile([PB, 3 * C_DIM], f32, tag="r3")
                r3v = r3[:].rearrange("p (i j) -> p i j", i=3)
                nc.gpsimd.tensor_tensor(r3v, r2v[:, 0:3, :], r2v[:, 3:6, :], op=OP.add)
                r4 = small.tile([PB, C_DIM], f32, tag="r4")
                nc.gpsimd.tensor_tensor(r4[:], r3[:, 0:C_DIM], r3[:, C_DIM:2 * C_DIM], op=OP.add)
                nc.gpsimd.tensor_tensor(vvs(t), r4[:], r3[:, 2 * C_DIM:3 * C_DIM], op=OP.add)

            xt_prev = None
            for t in [tt for _ in range(repeat) for tt in range(nrun)]:
                # ======== gate 1 (recurrence) ========
                xt_ps = make_xt(t)
                # prefetch the host-precomputed gates-aux logits
                zga = small.tile([PB, 49], f32, tag="zga")
                nc.sync.dma_start(zga[:], zgaux_d[:, t, :])

                zA = ps_za.tile([PB, NZA], f32, tag="zA")
                xbh = sbxh.tile([SLOT, PB], bf16, tag="xbh")
                nc.scalar.copy(xbh[:], xt_ps[:])
                nc.tensor.matmul(zA[:], xbh[:], wah[:], start=True, stop=False)
                nc.tensor.matmul(zA[:], xbh[:], wal[:], start=False, stop=True)
                zB = ps_zb.tile([PB, NZB], f32, tag="zB")
                nc.tensor.matmul(zB[:], xbh[:], wbh[:], start=True, stop=False)
                nc.tensor.matmul(zB[:], xbh[:], wbl[:], start=False, stop=True)

                E1 = sbe1.tile([PB, 576], f32, tag="E1")
                nc.scalar.activation(E1[:, 0:NZA], zA[:], AF.Exp)
                nc.scalar.activation(E1[:, NZA:576], zB[:, 0:64], AF.Exp)

                # gates: z = C-part (psum) + host aux logits
                zg = small.tile([PB, 49], f32, tag="zg")
                nc.vector.tensor_tensor(zg[:], zB[:, 64:NZB], zga[:], op=OP.add)
                nm = small.tile([PB, 1], f32, tag="nm")
                nc.vector.tensor_reduce(nm[:], zg[:, 0:C_DIM], axis=AX.X, op=OP.max, negate=True)
                Ep = small.tile([PB, C_DIM], f32, tag="Ep")
                Sp = small.tile([PB, 1], f32, tag="Sp")
                nc.scalar.activation(Ep[:], zg[:, 0:C_DIM], AF.Exp, bias=nm[:], accum_out=Sp[:])
                # cons cols pre-negated on host: one tanh covers
                # th = tanh(-zc/2) (cols 24..48) and tha = tanh(za/2) (col 48)
                thall = small.tile([PB, 25], f32, tag="thall")
                nc.scalar.activation(thall[:], zg[:, C_DIM:49], AF.Tanh, scale=0.5)
                th = thall[:, 0:C_DIM]
                tha = thall[:, C_DIM:25]

                # small chain kept on DVE, emitted before S1 so the gates
                # branch completes while the big reduce runs
                rSp = small.tile([PB, 1], f32, tag="rSp")
                nc.vector.reciprocal(rSp[:], Sp[:])
                acp = small.tile([PB, 1], f32, tag="acp")
                nc.vector.tensor_scalar(acp[:], tha, 1.0, sv(t, 32 + A_CPOT, 32 + A_CPOT + 1), op0=OP.add, op1=OP.mult)
                acprS = small.tile([PB, 1], f32, tag="acprS")
                nc.vector.tensor_scalar(acprS[:], acp[:], rSp[:], None, op0=OP.mult)
                Cpre = small.tile([PB, C_DIM], f32, tag="Cpre")
                nc.vector.scalar_tensor_tensor(Cpre[:], Ep[:], acprS[:], sv(t, 0, C_DIM), op0=OP.mult, op1=OP.add)
                Cp2 = small.tile([PB, C_DIM], f32, tag="Cp2")
                nc.vector.scalar_tensor_tensor(Cp2[:], th, 1.0, Cpre[:], op0=OP.add, op1=OP.mult)

                S1 = small.tile([PB, C_DIM], f32, tag="S1")
                nc.vector.tensor_reduce(S1[:], E1[:].rearrange("p (i j) -> p i j", i=C_DIM), axis=AX.X, op=OP.add)
                rS1 = small.tile([PB, C_DIM], f32, tag="rS1")
                nc.vector.reciprocal_approx_fast(rS1[:], S1[:])
                w1 = small.tile([PB, C_DIM], f32, tag="w1")
                nc.vector.scalar_tensor_tensor(w1[:], Cp2[:], 0.5, rS1[:], op0=OP.mult, op1=OP.mult)
                P1 = sbp1.tile([PB, 576], f32, tag="P1")
                nc.vector.tensor_tensor(
                    P1[:].rearrange("p (i j) -> p i j", i=C_DIM),
                    E1[:].rearrange("p (i j) -> p i j", i=C_DIM),
                    w1[:].unsqueeze(2).broadcast_to([PB, C_DIM, C_DIM]),
                    op=OP.mult,
                )
                nc.vector.tensor_reduce(
                    sv(t + 1, 0, C_DIM),
                    P1[:].rearrange("p (i j) -> p j i", i=C_DIM),
                    axis=AX.X, op=OP.add,
                )

                if xt_prev is not None:
                    with tc.high_priority(offset=-G2_PRIO):
                        gate2(max(t - 1, 0), xt_prev, xt_ps)
                xt_prev = xt_ps

            # last gate2 needs the transpose of the final slot
            xt_last = make_xt(nrun)
            with tc.high_priority(offset=-G2_PRIO):
                gate2(nrun - 1, xt_prev, xt_last)

            # outputs
            nc.sync.dma_start(ccell_d[:], sv3[:, 1:nstep + 1, 0:C_DIM])
            nc.sync.dma_start(cconv_d[:], stagev[:])

    nc.compile()
    return nc


# ---------------- host side ----------------

def _round_bf16(x):
    import ml_dtypes
    return x.astype(ml_dtypes.bfloat16).astype(np.float32)


def _host_prep(inputs, mm_mode):
    X = np.asarray(inputs["X"], np.float32)
    ORY = np.asarray(inputs["ORY"], np.float32)
    W_redis = np.asarray(inputs["W_redis"], np.float32)
    b_redis = np.asarray(inputs["b_redis"], np.float32)
    W_part = np.asarray(inputs["W_part"], np.float32)
    b_part = np.asarray(inputs["b_part"], np.float32)
    W_cons = np.asarray(inputs["W_cons"], np.float32)
    b_cons = np.asarray(inputs["b_cons"], np.float32)
    W_assim = np.asarray(inputs["W_assim"], np.float32)
    b_assim = np.asarray(inputs["b_assim"], np.float32)

    # preprocessing (mirrors reference, fp32)
    N_cum = np.cumsum(X[:, :, 3], axis=-1, dtype=np.float32)[..., None]
    t_ave = ((X[:, :, 1:2] + X[:, :, 2:3]) * 0.5).astype(np.float32)
    dvs = ORY[:, :, 0:1]
    rad = X[:, :, 0:1]
    tmax = X[:, :, 1:2]
    tmin = X[:, :, 2:3]
    FRPAR = np.float32(0.5)
    eff = (np.float32(0.54)
           - (np.clip(t_ave * np.float32(50.0), np.float32(10.0), np.float32(40.0))
              - np.float32(10.0)) / np.float32(30.0) * np.float32(0.54 - 0.36))
    R2C = (np.float32(40000.0 / 20000.0) * FRPAR * eff / np.float32(3.6)
           * np.float32(12.0 / 44.0))
    cpot = (rad * R2C).astype(np.float32)

    rad_h = _round_bf16(rad)
    rad_l = (rad - rad_h).astype(np.float32)

    aux8 = np.zeros((B, T, 8), np.float32)
    aux8[:, :, A_DVS] = dvs[:, :, 0]
    aux8[:, :, A_RADH] = rad_h[:, :, 0]
    aux8[:, :, A_RADL] = rad_l[:, :, 0]
    aux8[:, :, A_TMAX] = tmax[:, :, 0]
    aux8[:, :, A_TMIN] = tmin[:, :, 0]
    aux8[:, :, A_NCUM] = N_cum[:, :, 0]
    aux8[:, :, A_ONE] = 1.0
    aux8[:, :, A_CPOT] = cpot[:, :, 0] * 0.5

    # fused weight matrices, row layout = x rows:
    # [C(24), pad(8), dvs, rad_h, rad_l, tmax, tmin, N_cum, ones, cpot/2]
    def rowstack(Wt, bias, masked):
        out = np.zeros((SLOT, Wt.shape[1]), np.float32)
        out[0:C_DIM] = Wt[0:C_DIM]
        if masked:
            out[32 + A_RADH] = Wt[25]
            out[32 + A_RADL] = Wt[25]
        else:
            out[32 + A_DVS] = Wt[24]
            out[32 + A_RADH] = Wt[25]
            out[32 + A_RADL] = Wt[25]
            out[32 + A_TMAX] = Wt[26]
            out[32 + A_TMIN] = Wt[27]
            out[32 + A_NCUM] = Wt[28]
        out[32 + A_ONE] = bias
        return out

    Wr = rowstack(W_redis.T, b_redis, masked=True)          # [40, 576]

    # gates: C-part on device (rows 0..23), aux part precomputed on host.
    # cons block negated so one tanh(0.5*z) op serves cons' and assim.
    Wg = np.concatenate([W_part.T, -W_cons.T, W_assim.T], axis=1)  # [29, 49]
    WgC = np.zeros((SLOT, 49), np.float32)
    WgC[0:C_DIM] = Wg[0:C_DIM]
    bg = np.concatenate([b_part, -b_cons, b_assim]).astype(np.float64)
    # aux logits: [dvs, rad, tmax, tmin, N_cum] @ Wg_aux + bias (float64)
    auxfull = np.stack([dvs[:, :, 0], rad[:, :, 0], tmax[:, :, 0],
                        tmin[:, :, 0], N_cum[:, :, 0]], axis=-1).astype(np.float64)
    Zgaux = (auxfull @ Wg[C_DIM:IN_DIM].astype(np.float64) + bg).astype(np.float32)
    Zgaux = np.ascontiguousarray(Zgaux[:, 0:T - 1, :])      # [B, 365, 49]

    WA = np.ascontiguousarray(Wr[:, 0:NZA])                 # [40, 512]
    WBtail = np.ascontiguousarray(Wr[:, NZA:576])           # [40, 64]
    WB = np.concatenate([WBtail, WgC], axis=1)              # [40, 113]

    WAh = _round_bf16(WA)
    WAl = _round_bf16(WA - WAh)
    WBh = _round_bf16(WB)
    WBl = _round_bf16(WB - WBh)
    ident = np.eye(PB, dtype=np.float32)
    return dict(aux8=aux8, zgaux=Zgaux, wah=WAh, wal=WAl, wbh=WBh, wbl=WBl,
                ident=ident)


_CACHE = {}


def _get_bass(mm_mode):
    key = (NSTEP, mm_mode)
    if key not in _CACHE:
        _CACHE[key] = _build_bass(NSTEP, mm_mode)
    return _CACHE[key]


def kernel(**inputs):
    from concourse.bass_utils import run_bass_kernel_spmd

    mm_mode = inputs.pop("_mm_mode", "hilo")
    trace = inputs.pop("_trace", False)

    hp = _host_prep(inputs, mm_mode)
    nc = _get_bass(mm_mode)

    in_maps = []
    for c in range(NCORES):
        sl = slice(c * PB, (c + 1) * PB)
        m = dict(hp)
        m["aux8"] = np.ascontiguousarray(hp["aux8"][sl])
        m["zgaux"] = np.ascontiguousarray(hp["zgaux"][sl])
        in_maps.append(m)

    import concourse.mybir as _mybir
    expected = set()
    for alloc in nc.m.functions[0].allocations:
        if isinstance(alloc, _mybir.MemoryLocationSet) and alloc.kind == "ExternalInput":
            expected.add(alloc.memorylocations[0].name)
    in_maps = [{k: v for k, v in m.items() if k in expected} for m in in_maps]
    res = run_bass_kernel_spmd(nc, in_maps, core_ids=list(range(NCORES)),
                               trace=trace)
    ccell = np.concatenate([r["ccell"] for r in res.results], axis=0)  # [B, 365, 24]
    cconv = np.concatenate([r["cconv"] for r in res.results], axis=0)

    # host postprocessing: summary channels + all_day assembly
    ORY = np.asarray(inputs["ORY"], np.float32)
    c2a = np.asarray(inputs["c2a_par"], np.float32)
    g2y = np.asarray(inputs["g2y_par"], np.float32)
    pai = np.abs(ccell * c2a).sum(2, keepdims=True)
    lea = ccell[:, :, 0:LEA].sum(2, keepdims=True) / np.float32(0.419)
    ste = ccell[:, :, LEA:LEA + STE].sum(2, keepdims=True) / np.float32(0.431)
    gra = ccell[:, :, LEA + STE:C_DIM].sum(2, keepdims=True) / np.float32(0.487)
    agb = lea + ste + gra
    yie = np.abs(ccell[:, :, LEA + STE:C_DIM] * g2y).sum(2, keepdims=True) / np.float32(0.487)
    dvs = ORY[:, :, 0:1]
    all_day = np.concatenate([dvs[:, 1:], pai, lea, ste, gra, agb, yie], axis=2)
    all_day = np.concatenate([ORY[:, 0:1, :], all_day], axis=1)
    if "_results_holder" in inputs:
        inputs["_results_holder"].append(res)
    return all_day.astype(np.float32), ccell.astype(np.float32), cconv.astype(np.float32)
